# revision 1
# baseline (speedup 1.0000x reference)
"""CRF loss (log-likelihood) kernel for Trainium2, 8 NeuronCores.

Strategy:
  - Data-parallel: batch 512 sharded as 64 per core.
  - Denominator (forward algorithm): exp-space scans. Forward scan over
    t=0..383 and backward scan over t=767..384 run concurrently (two
    independent serial chains), meeting in the middle. Per step: one
    32x32xB matmul with stationary exp(T) weights + one elementwise
    multiply by exp(X_t). PE-array band cycling (tile_position) lets the
    transposed exp(X) tiles be consumed directly from 4-row-band blocks.
  - Renormalization every 8 steps (sum-based, reciprocal-approx), applied
    two steps late so it stays off the serial critical path; the scale
    factors are logged in bulk at the end.
  - Numerator: GPSIMD ap_gather. Emission gather uses a t-mod-16 wrapped
    layout (gather indices are shared per 16-partition group, so batch b
    owns one group and its timesteps are spread across the 16 partitions).
    Transition/start/end gather reads a replicated 1092-entry table.
"""

import os
import sys

import numpy as np

for _p in ("/opt/trn_rl_repo", "/root/.axon_site/_ro/trn_rl_repo"):
    if os.path.isdir(_p) and _p not in sys.path:
        sys.path.insert(0, _p)

BS, T, NTAG = 512, 768, 32
NCORES = 8
B = BS // NCORES  # 64 batch per core
HALF = 384  # forward scan covers t=0..383, backward t=767..384
RENORM = 8

_state = {}
_DEBUG = False


def _emit(tc, nc, aps):
    import concourse.bass as bass
    from concourse import masks, mybir

    f32 = mybir.dt.float32
    i32 = mybir.dt.int32
    i16 = mybir.dt.int16
    AF = mybir.ActivationFunctionType
    ALU = mybir.AluOpType
    AX = mybir.AxisListType

    Xd, Yd, Td, Sd, Ed, Od = aps
    Xf = Xd.rearrange("b t j -> b (t j)")  # [64, 24576]

    ctx = tc.octx if hasattr(tc, "octx") else None
    from contextlib import ExitStack

    es = _state["es"] = ExitStack()
    persist = es.enter_context(tc.tile_pool(name="persist", bufs=1))
    xin = es.enter_context(tc.tile_pool(name="xin", bufs=4))
    prep_ps = es.enter_context(tc.tile_pool(name="prep_ps", bufs=2, space="PSUM"))
    fwd_ps = es.enter_context(tc.tile_pool(name="fwd_ps", bufs=2, space="PSUM"))
    bwd_ps = es.enter_context(tc.tile_pool(name="bwd_ps", bufs=2, space="PSUM"))
    s_ps = es.enter_context(tc.tile_pool(name="s_ps", bufs=2, space="PSUM"))
    scratch = es.enter_context(tc.tile_pool(name="scratch", bufs=2))
    rbpool = es.enter_context(tc.tile_pool(name="rb", bufs=3))
    gpool = es.enter_context(tc.tile_pool(name="gout", bufs=2))

    # ---------------- Phase A: constants ----------------
    ident = persist.tile([64, 64], f32)
    masks.make_identity(nc, ident[:])

    ttab = persist.tile([32, 32], f32)
    nc.sync.dma_start(ttab[:], Td)
    exT4 = persist.tile([128, 32], f32)   # exp(T) replicated on 4 bands
    exTT4 = persist.tile([128, 32], f32)  # exp(T)^T replicated on 4 bands
    nc.scalar.activation(exT4[0:32, :], ttab[:], AF.Exp)
    tps = prep_ps.tile([32, 32], f32, tag="pp")
    nc.tensor.transpose(tps[:], ttab[:], ident[0:32, 0:32])
    nc.scalar.activation(exTT4[0:32, :], tps[:], AF.Exp)
    for bnd in (1, 2, 3):
        nc.sync.dma_start(exT4[32 * bnd:32 * bnd + 32, :], exT4[0:32, :])
        nc.sync.dma_start(exTT4[32 * bnd:32 * bnd + 32, :], exTT4[0:32, :])

    ones4 = persist.tile([128, 1], f32)
    nc.vector.memset(ones4[:], 1.0)

    sraw = persist.tile([128, 1], f32)
    nc.sync.dma_start(sraw[0:32, :], Sd)
    nc.sync.dma_start(sraw[96:128, :], Ed)
    expSE = persist.tile([128, 1], f32)  # exp(start) on band0, exp(end) on band3
    nc.scalar.activation(expSE[0:32, :], sraw[0:32, :], AF.Exp)
    nc.scalar.activation(expSE[96:128, :], sraw[96:128, :], AF.Exp)

    # ---------------- Phase B: EX = exp(X)^T blocks ----------------
    # EX[:, 64k:64k+64] band r holds exp(X[:, 4k+r, :])^T as [32 j, 64 b]
    EX = persist.tile([128, 64 * (T // 4)], f32)
    for k in range(T // 4):
        xb = xin.tile([64, 128], f32)
        nc.sync.dma_start(xb[:], Xf[:, 128 * k:128 * k + 128])
        tp = prep_ps.tile([128, 64], f32, tag="pp")
        nc.tensor.transpose(tp[:], xb[:], ident[:])
        nc.scalar.activation(EX[:, 64 * k:64 * k + 64], tp[:], AF.Exp)

    def ex_slice(t):
        bnd, k = t % 4, t // 4
        return EX[32 * bnd:32 * bnd + 32, 64 * k:64 * k + 64]

    # ---------------- Phase C: scans ----------------
    U = persist.tile([128, 64], f32)
    W = persist.tile([128, 64], f32)
    NREN = 2 * len(range(RENORM, 377, RENORM)) + 2
    rst = persist.tile([1, 64 * NREN], f32)

    # u_0 = exp(start) * ex_0 on band 0
    nc.vector.tensor_scalar_mul(U[0:32, :], ex_slice(0), expSE[0:32, 0:1])
    # w_767 = exp(end) on band 3, replicated along batch
    onesb = persist.tile([128, 64], f32)
    nc.vector.memset(onesb[96:128, :], 1.0)
    nc.vector.tensor_scalar_mul(W[96:128, :], onesb[96:128, :], expSE[96:128, 0:1])

    fwd_apply = {}  # t -> rb tile (apply at fwd step t)
    bwd_apply = {}  # t -> rb tile (apply at bwd z-step t)
    ren_slot = [0]

    def band(t):
        return 32 * (t % 4)

    def emit_renorm(state, t_apply, pending):
        # state band 0 holds the tile to measure; scale applied at t_apply
        sp = s_ps.tile([1, 64], f32, tag="sp")
        nc.tensor.matmul(sp[:], ones4[0:32, 0:1], state[0:32, :],
                         tile_position=(0, 0))
        m = ren_slot[0]
        ren_slot[0] += 1
        rsl = rst[0:1, 64 * m:64 * m + 64]
        nc.vector.reciprocal_approx_fast(rsl, sp[:])
        rb = rbpool.tile([128, 64], f32)
        nc.gpsimd.partition_broadcast(rb[:], rsl)
        pending[t_apply] = rb

    for r in range(HALF - 1):  # r = 0..382
        # ---- forward step t = r+1: u_t = (exT^T u_{t-1}) * ex_t ----
        t = r + 1
        bp, bt = band(t - 1), band(t)
        vp = fwd_ps.tile([128, 64], f32, tag="fp")
        nc.tensor.matmul(vp[bt:bt + 32, :], exT4[bp:bp + 32, :], U[bp:bp + 32, :],
                         tile_position=(bp, bt))
        if t in fwd_apply:
            rb = fwd_apply.pop(t)
            tmp = scratch.tile([128, 64], f32)
            nc.vector.tensor_mul(tmp[bt:bt + 32, :], vp[bt:bt + 32, :],
                                 rb[bt:bt + 32, :])
            nc.vector.tensor_mul(U[bt:bt + 32, :], tmp[bt:bt + 32, :], ex_slice(t))
        else:
            nc.vector.tensor_mul(U[bt:bt + 32, :], vp[bt:bt + 32, :], ex_slice(t))
        if t % RENORM == 0 and t <= 376:
            emit_renorm(U, t + 2, fwd_apply)

        # ---- backward step: z_tb = w_tb * ex_tb ; w_{tb-1} = exp(T) z_tb ----
        tb = 767 - r
        bz, bo = band(tb), band(tb - 1)
        z = scratch.tile([128, 64], f32)
        if tb in bwd_apply:
            rb = bwd_apply.pop(tb)
            tmp = scratch.tile([128, 64], f32)
            nc.vector.tensor_mul(tmp[bz:bz + 32, :], W[bz:bz + 32, :],
                                 rb[bz:bz + 32, :])
            nc.vector.tensor_mul(z[bz:bz + 32, :], tmp[bz:bz + 32, :], ex_slice(tb))
        else:
            nc.vector.tensor_mul(z[bz:bz + 32, :], W[bz:bz + 32, :], ex_slice(tb))
        wp = bwd_ps.tile([128, 64], f32, tag="bp")
        nc.tensor.matmul(wp[bo:bo + 32, :], exTT4[bz:bz + 32, :], z[bz:bz + 32, :],
                         tile_position=(bz, bo))
        nc.vector.tensor_copy(W[bo:bo + 32, :], wp[bo:bo + 32, :])
        tw = tb - 1  # index of the w just produced
        if tw % RENORM == 0 and 392 <= tw <= 760:
            emit_renorm(W, tw - 2, bwd_apply)

    # ---- final renorms so the combine stays within the Ln table range ----
    def final_renorm(state, bnd):
        sp = s_ps.tile([1, 64], f32, tag="sp")
        nc.tensor.matmul(sp[:], ones4[bnd:bnd + 32, 0:1], state[bnd:bnd + 32, :],
                         tile_position=(bnd, 0))
        m = ren_slot[0]
        ren_slot[0] += 1
        rsl = rst[0:1, 64 * m:64 * m + 64]
        nc.vector.reciprocal_approx_fast(rsl, sp[:])
        rb = rbpool.tile([128, 64], f32)
        nc.gpsimd.partition_broadcast(rb[:], rsl)
        nc.vector.tensor_mul(state[bnd:bnd + 32, :], state[bnd:bnd + 32, :],
                             rb[bnd:bnd + 32, :])

    final_renorm(U, 96)  # u_383 lives on band 3
    final_renorm(W, 0)   # w_384 lives on band 0

    # ---- combine at the middle: logZ = ln(sum_j (exT^T u_383)_j * z_384_j) + c
    qp = fwd_ps.tile([128, 64], f32, tag="fp")
    nc.tensor.matmul(qp[0:32, :], exT4[96:128, :], U[96:128, :],
                     tile_position=(96, 0))
    z384 = scratch.tile([128, 64], f32)
    nc.vector.tensor_mul(z384[0:32, :], W[0:32, :], ex_slice(384))
    qz = scratch.tile([128, 64], f32)
    nc.vector.tensor_mul(qz[0:32, :], qp[0:32, :], z384[0:32, :])
    combo = s_ps.tile([1, 64], f32, tag="sp")
    nc.tensor.matmul(combo[:], ones4[0:32, 0:1], qz[0:32, :], tile_position=(0, 0))

    # ---------------- Phase D: numerator (gathers) ----------------
    NQ = T // 16  # 48 wrapped columns
    # Y in wrapped layout [p=t%16, c=t//16] per batch group
    Ywr = persist.tile([128, 8 * NQ], i32)
    for b in range(B):
        g, tau = b % 8, b // 8
        nc.sync.dma_start(
            Ywr[16 * g:16 * g + 16, NQ * tau:NQ * tau + NQ],
            Yd[b:b + 1, :].rearrange("a (c p) -> a p c", p=16),
        )
    Ywrf = persist.tile([128, 8 * NQ], f32)
    nc.vector.tensor_copy(Ywrf[:], Ywr[:])
    iow = persist.tile([128, 8 * NQ], i16)
    nc.gpsimd.iota(iow[:], pattern=[[0, 8], [32, NQ]], base=0, channel_multiplier=0)
    iowf = persist.tile([128, 8 * NQ], f32)
    nc.vector.tensor_copy(iowf[:], iow[:])
    eidxf = persist.tile([128, 8 * NQ], f32)
    nc.vector.tensor_add(eidxf[:], iowf[:], Ywrf[:])
    EIDX = persist.tile([128, 8 * NQ], i16)
    nc.vector.tensor_copy(EIDX[:], eidxf[:])

    # X data in wrapped layout: [p=t%16, (t//16)*32 + j] per batch group
    XW = []
    for tau in range(8):
        xw = persist.tile([128, NQ * 32], f32)
        XW.append(xw)
        for g in range(8):
            b = 8 * tau + g
            nc.sync.dma_start(
                xw[16 * g:16 * g + 16, :],
                Xf[b:b + 1, :].rearrange("a (q p j) -> a p q j", p=16, j=32),
            )

    # flat Y for pair indices
    Yi = persist.tile([64, T], i32)
    nc.sync.dma_start(Yi[:], Yd)
    Yf_ = persist.tile([64, T], f32)
    nc.vector.tensor_copy(Yf_[:], Yi[:])
    NP = 800  # 767 pairs + start + end + 31 pad (16-mult, 4B-aligned wrap)
    pidx = persist.tile([64, NP], f32)
    nc.vector.scalar_tensor_tensor(pidx[:, 0:767], Yf_[:, 0:767], 32.0,
                                   Yf_[:, 1:768], op0=ALU.mult, op1=ALU.add)
    nc.vector.tensor_scalar_add(pidx[:, 767:768], Yf_[:, 0:1], 1024.0)
    nc.vector.tensor_scalar_add(pidx[:, 768:769], Yf_[:, 767:768], 1056.0)
    nc.vector.memset(pidx[:, 769:800], 1088.0)
    pidx16 = persist.tile([64, NP], i16)
    nc.vector.tensor_copy(pidx16[:], pidx[:])
    dpool = _state["es"].enter_context(tc.tile_pool(name="dram", bufs=1,
                                                   space="DRAM"))
    from concourse.tile import add_dep_helper

    pd = dpool.tile([64, NP], i16)
    pdw = nc.sync.dma_start(pd[:], pidx16[:])
    NPC = NP // 16  # 50
    PIDX = persist.tile([128, 8 * NPC], i16)
    for b in range(B):
        g, tau = b % 8, b // 8
        wi = nc.sync.dma_start(
            PIDX[16 * g:16 * g + 16, NPC * tau:NPC * tau + NPC],
            pd[b:b + 1, :].rearrange("a (c p) -> a p c", p=16),
        )
        add_dep_helper(wi.ins, pdw.ins, sync=True,
                       reason="wrap read waits for dram roundtrip write")

    # table: [T flat 1024 | start 32 | end 32 | zeros 4] replicated to 128 parts
    TTAB = persist.tile([128, 1092], f32)
    nc.gpsimd.memset(TTAB[0:1, :], 0.0)
    nc.sync.dma_start(TTAB[0:1, 0:1024], Td.rearrange("i j -> (i j)"))
    nc.sync.dma_start(TTAB[0:1, 1024:1056], Sd)
    nc.sync.dma_start(TTAB[0:1, 1056:1088], Ed)
    nc.gpsimd.partition_broadcast(TTAB[:], TTAB[0:1, :])

    # static diag mask for the emission gather: [p, k] = (k%16 == p%16)
    iok = persist.tile([128, T], i16)
    nc.gpsimd.iota(iok[:], pattern=[[0, NQ], [1, 16]], base=0, channel_multiplier=0)
    iokf = persist.tile([128, T], f32)
    nc.vector.tensor_copy(iokf[:], iok[:])
    iop = persist.tile([128, 1], i16)
    nc.gpsimd.iota(iop[:], pattern=[[0, 1]], base=0, channel_multiplier=1)
    pmod = persist.tile([128, 1], i16)
    nc.vector.tensor_scalar(pmod[:], iop[:], 15, None, op0=ALU.bitwise_and)
    pmodf = persist.tile([128, 1], f32)
    nc.vector.tensor_copy(pmodf[:], pmod[:])
    dmask = persist.tile([128, T], f32)
    nc.vector.tensor_scalar(dmask[:], iokf[:], pmodf[:], None, op0=ALU.is_equal)

    # selection matrices for the per-group combine matmuls
    iog = persist.tile([128, 8], i16)
    nc.gpsimd.iota(iog[:], pattern=[[1, 8]], base=0, channel_multiplier=0)
    iogf = persist.tile([128, 8], f32)
    nc.vector.tensor_copy(iogf[:], iog[:])
    pdiv = persist.tile([128, 1], i16)
    nc.vector.tensor_scalar(pdiv[:], iop[:], 4, None, op0=ALU.logical_shift_right)
    pdivf = persist.tile([128, 1], f32)
    nc.vector.tensor_copy(pdivf[:], pdiv[:])
    SELe = persist.tile([128, 8], f32)
    nc.vector.tensor_scalar(SELe[:], iogf[:], pdivf[:], None, op0=ALU.is_equal)
    SELt = persist.tile([128, 8], f32)
    nc.vector.tensor_scalar_mul(SELt[:], SELe[:], 1.0 / 16.0)

    empart = persist.tile([128, 8], f32)
    tpart = persist.tile([128, 8], f32)
    for tau in range(8):
        go = gpool.tile([128, T], f32)
        nc.gpsimd.ap_gather(go[:], XW[tau][:], EIDX[:, NQ * tau:NQ * tau + NQ],
                            channels=128, num_elems=NQ * 32, d=1, num_idxs=T)
        junk = gpool.tile([128, T], f32)
        nc.vector.scalar_tensor_tensor(junk[:], go[:], 1.0, dmask[:],
                                       op0=ALU.bypass, op1=ALU.mult,
                                       accum_out=empart[:, tau:tau + 1])
        to = gpool.tile([128, NP], f32)
        nc.gpsimd.ap_gather(to[:], TTAB[:], PIDX[:, NPC * tau:NPC * tau + NPC],
                            channels=128, num_elems=1092, d=1, num_idxs=NP)
        nc.vector.tensor_reduce(tpart[:, tau:tau + 1], to[:], AX.X, ALU.add)

    nump = prep_ps.tile([1, 64], f32, tag="pp")
    for tau in range(8):
        sl = nump[0:1, 8 * tau:8 * tau + 8]
        nc.tensor.matmul(sl, empart[:, tau:tau + 1], SELe[:], start=True,
                         stop=False, tile_position=(0, 0))
        nc.tensor.matmul(sl, tpart[:, tau:tau + 1], SELt[:], start=False,
                         stop=True, tile_position=(0, 0))

    # ---------------- Phase E: final assembly ----------------
    lncombo = persist.tile([1, 64], f32)
    nc.scalar.activation(lncombo[:], combo[:], AF.Ln)
    lnr = persist.tile([1, 64 * NREN], f32)
    nc.scalar.activation(lnr[:], rst[:], AF.Ln)
    lnrsum = persist.tile([1, 64], f32)
    nc.vector.tensor_reduce(lnrsum[:], lnr[:].rearrange("p (m b) -> p b m", b=64),
                            AX.X, ALU.add)
    f1 = persist.tile([1, 64], f32)
    nc.vector.tensor_sub(f1[:], nump[:], lncombo[:])
    f2 = persist.tile([1, 64], f32)
    nc.vector.tensor_add(f2[:], f1[:], lnrsum[:])
    nc.sync.dma_start(Od, f2[:])

    if _DEBUG:
        def dout(name, ap):
            d = nc.dram_tensor(name, list(ap.shape), ap.dtype,
                               kind="ExternalOutput").ap()
            nc.sync.dma_start(d, ap)
        dout("d_empart", empart[:]); dout("d_tpart", tpart[:])
        dout("d_eidx", EIDX[:]); dout("d_pidx", PIDX[:])
        dout("d_dmask", dmask[:]); dout("d_xw0", XW[0][:])
        dout("d_ttab", TTAB[:]); dout("d_ywr", Ywr[:])
        dout("d_rst", rst[:]); dout("d_u", U[:]); dout("d_w", W[:])
        dout("d_sele", SELe[:]); dout("d_lncombo", lncombo[:])
        dout("d_lnrsum", lnrsum[:]); dout("d_nump_sb", f1[:])
        dout("d_yi", Yi[:]); dout("d_pid16", pidx16[:])

    es.close()


def _build():
    import concourse.tile as tile
    from concourse import bacc, mybir

    f32 = mybir.dt.float32
    i32 = mybir.dt.int32

    nc = bacc.Bacc("TRN2", target_bir_lowering=False, debug=False,
                   enable_asserts=False, num_devices=NCORES)
    Xd = nc.dram_tensor("x", [B, T, NTAG], f32, kind="ExternalInput").ap()
    Yd = nc.dram_tensor("y", [B, T], i32, kind="ExternalInput").ap()
    Td = nc.dram_tensor("t", [NTAG, NTAG], f32, kind="ExternalInput").ap()
    Sd = nc.dram_tensor("s", [NTAG], f32, kind="ExternalInput").ap()
    Ed = nc.dram_tensor("e", [NTAG], f32, kind="ExternalInput").ap()
    Od = nc.dram_tensor("o", [B], f32, kind="ExternalOutput").ap()
    with tile.TileContext(nc) as tc:
        _emit(tc, nc, (Xd, Yd, Td, Sd, Ed, Od))
    nc.compile()
    return nc


def _numpy_fallback(X, Y, mask, transition, start_trans, end_trans):
    X = np.asarray(X, np.float64)
    Y = np.asarray(Y, np.int64)
    m = np.asarray(mask, bool)
    Tm = np.asarray(transition, np.float64)
    st = np.asarray(start_trans, np.float64)
    en = np.asarray(end_trans, np.float64)
    bs, sl, nt = X.shape
    rb = np.arange(bs)
    mf = m.astype(np.float64)
    score = st[Y[:, 0]] + X[rb, 0, Y[:, 0]]
    emit = np.take_along_axis(X[:, 1:], Y[:, 1:, None], axis=2)[..., 0]
    tr = Tm[Y[:, :-1], Y[:, 1:]]
    score = score + np.sum((tr + emit) * mf[:, 1:], axis=1)
    each_len = m.sum(1).astype(np.int64)
    last_tag = Y[rb, each_len - 1]
    score = score + en[last_tag] * mf[rb, each_len - 1]
    alpha = st[None, :] + X[:, 0]
    for t in range(1, sl):
        s = alpha[:, :, None] + Tm[None] + X[:, t][:, None, :]
        mx = s.max(1)
        new = mx + np.log(np.exp(s - mx[:, None, :]).sum(1))
        alpha = np.where(m[:, t][:, None], new, alpha)
    mx = (alpha + en).max(1)
    logZ = mx + np.log(np.exp(alpha + en - mx[:, None]).sum(1))
    return (score - logZ).astype(np.float32)


def kernel(X, Y, mask, transition, start_trans, end_trans):
    X = np.ascontiguousarray(np.asarray(X, dtype=np.float32))
    Yc = np.ascontiguousarray(np.asarray(Y).astype(np.int32))
    Tm = np.ascontiguousarray(np.asarray(transition, dtype=np.float32))
    st = np.ascontiguousarray(np.asarray(start_trans, dtype=np.float32))
    en = np.ascontiguousarray(np.asarray(end_trans, dtype=np.float32))
    mk = np.asarray(mask)

    if X.shape != (BS, T, NTAG) or not bool(mk.all()):
        return _numpy_fallback(X, Y, mask, transition, start_trans, end_trans)

    from concourse import bass_utils

    if "nc" not in _state:
        _state["nc"] = _build()
    nc = _state["nc"]

    in_maps = []
    for c in range(NCORES):
        sl = slice(B * c, B * (c + 1))
        in_maps.append({"x": X[sl], "y": Yc[sl], "t": Tm, "s": st, "e": en})
    res = bass_utils.run_bass_kernel_spmd(nc, in_maps, core_ids=list(range(NCORES)))
    out = np.concatenate([res.results[c]["o"] for c in range(NCORES)])
    return out.astype(np.float32)


if __name__ == "__main__":
    sys.path.insert(0, "/root/problem")
    import reference

    inputs = reference.setup_inputs()
    inputs = {k: np.asarray(v) for k, v in inputs.items()}
    exp = np.asarray(reference.reference(**inputs))
    act = kernel(**inputs)
    err = np.abs(act - exp) / np.maximum(np.abs(exp), 1e-6)
    print("max rel err:", err.max(), "mean:", err.mean())



# revision 24
# speedup vs baseline: 2.1150x; 2.1150x over previous
"""CRF loss (log-likelihood) kernel for Trainium2, 8 NeuronCores.

Strategy (v2):
  - Data-parallel: batch 512 sharded as 64 per core.
  - Denominator: exp-space forward+backward scans MERGED into one serial
    chain of 64x64 bf16 matmuls (block-diagonal weights [[exp(T),0],
    [0,exp(T)^T]]) + one DVE multiply per step.  Chains meet in the
    middle (384 steps).  Emissions are pre-exponentiated into a paired
    layout EX[pair r] = [exp(x_r)^T ; exp(x_{767-r})^T] so each step
    consumes a single contiguous [64,64] slice.
  - Renormalization every 8 steps, fully off the critical path: one
    ones-matmul measures both chain sums, DVE reciprocal, one PE
    outer-product broadcast, one DVE multiply pre-scales the EX slice
    consumed 4 steps later.  Scales are logged and folded back via Ln at
    the end.
  - Emission prep (DMA fp32 chunk -> ACT exp-cast to bf16 with the bwd
    half time-reversed via negative-stride reads -> PE transpose ->
    ACT copy to SBUF) is pipelined one chunk ahead of the scan, with
    transposes interleaved one-per-step so the in-order PE queue never
    blocks the chain.
  - Numerator: GPSIMD ap_gather exactly as v1, but all wrapped-layout
    DMAs are batched into single rearranged descriptors and the gathers
    are issued early so they stream concurrently with the scan.
"""

import os
import sys

import numpy as np

for _p in ("/opt/trn_rl_repo", "/root/.axon_site/_ro/trn_rl_repo"):
    if os.path.isdir(_p) and _p not in sys.path:
        sys.path.insert(0, _p)

BS, T, NTAG = 512, 768, 32
NCORES = 8
B = BS // NCORES  # 64 batch per core
NPAIR = T // 2  # 384 merged scan steps
CH = 32         # pairs per pipeline chunk
NCHUNK = NPAIR // CH  # 12
RENORM = 8
REN_LAG = 4     # renorm measured at r applied to EX pair r+REN_LAG

_state = {}
_DEBUG = False


def _emit(tc, nc, aps):
    import concourse.bass as bass
    from concourse import masks, mybir
    from concourse.tile import add_dep_helper

    f32 = mybir.dt.float32
    bf16 = mybir.dt.bfloat16
    i32 = mybir.dt.int32
    i16 = mybir.dt.int16
    AF = mybir.ActivationFunctionType
    ALU = mybir.AluOpType
    AX = mybir.AxisListType

    Xd, Yd, Td, Sd, Ed, Od = aps
    Xf = Xd.rearrange("b t j -> b (t j)")  # [64, 24576]

    from contextlib import ExitStack

    es = _state["es"] = ExitStack()
    persist = es.enter_context(tc.tile_pool(name="persist", bufs=1))
    xin = es.enter_context(tc.tile_pool(name="xin", bufs=3))
    u2pool = es.enter_context(tc.tile_pool(name="u2", bufs=3))
    sc_ps = es.enter_context(tc.tile_pool(name="sc_ps", bufs=3, space="PSUM"))
    tp_ps = es.enter_context(tc.tile_pool(name="tp_ps", bufs=2, space="PSUM"))
    s_ps = es.enter_context(tc.tile_pool(name="s_ps", bufs=1, space="PSUM"))
    rb_ps = es.enter_context(tc.tile_pool(name="rb_ps", bufs=1, space="PSUM"))
    pp_ps = es.enter_context(tc.tile_pool(name="pp_ps", bufs=1, space="PSUM"))
    gpool = es.enter_context(tc.tile_pool(name="gout", bufs=2))

    # ---------------- constants ----------------
    ident = persist.tile([128, 64], bf16)   # identity at partitions 64-127
    masks.make_identity(nc, ident[64:128, :])
    id32 = persist.tile([32, 32], f32)
    masks.make_identity(nc, id32[:])

    ttab = persist.tile([32, 32], f32)
    nc.sync.dma_start(ttab[:], Td)
    # W64 = [[exp(T), 0], [0, exp(T)^T]]  (bf16, partitions 0-63)
    W64 = persist.tile([64, 64], bf16)
    nc.vector.memset(W64[:], 0.0)
    nc.scalar.activation(W64[0:32, 0:32], ttab[:], AF.Exp)
    # exp(T)^T: transpose at partition 0 (transpose outputs must land at
    # PSUM partition 0), exp there, then DMA the result to partitions
    # 32-63 for the bwd weight block and the final combine matmul.
    tps2 = pp_ps.tile([32, 32], f32, tag="pp")
    nc.tensor.transpose(tps2[:], ttab[:], id32[:])
    exTT0 = persist.tile([32, 32], bf16)
    nc.scalar.activation(exTT0[:], tps2[:], AF.Exp)
    nc.sync.dma_start(W64[32:64, 32:64], exTT0[:])
    exTT32 = persist.tile([64, 32], bf16)
    nc.sync.dma_start(exTT32[32:64, :], exTT0[:])

    sraw = persist.tile([64, 1], f32)
    nc.sync.dma_start(sraw[0:32, :], Sd)
    nc.sync.dma_start(sraw[32:64, :], Ed)
    expSE = persist.tile([64, 1], f32)  # exp(start) rows 0-31, exp(end) 32-63
    nc.scalar.activation(expSE[:], sraw[:], AF.Exp)

    # sum-selector for renorm: col0 sums rows 0-31 (u), col1 rows 32-63 (y)
    ones2 = persist.tile([64, 2], bf16)
    nc.vector.memset(ones2[:], 0.0)
    nc.vector.memset(ones2[0:32, 0:1], 1.0)
    nc.vector.memset(ones2[32:64, 1:2], 1.0)
    # broadcast selector: row0 -> partitions 0-31, row1 -> partitions 32-63
    # (built via iota+compare; sub-32-aligned partition writes are illegal)
    itc = persist.tile([2, 64], i16)
    nc.gpsimd.iota(itc[:], pattern=[[1, 64]], base=0, channel_multiplier=0)
    itcs = persist.tile([2, 64], i16)
    nc.vector.tensor_scalar(itcs[:], itc[:], 5, None, op0=ALU.logical_shift_right)
    itcsf = persist.tile([2, 64], f32)
    nc.vector.tensor_copy(itcsf[:], itcs[:])
    itp = persist.tile([2, 1], i16)
    nc.gpsimd.iota(itp[:], pattern=[[0, 1]], base=0, channel_multiplier=1)
    itpf = persist.tile([2, 1], f32)
    nc.vector.tensor_copy(itpf[:], itp[:])
    SEL2T = persist.tile([2, 64], bf16)
    nc.vector.tensor_scalar(SEL2T[:], itcsf[:], itpf[:], None, op0=ALU.is_equal)
    ones2c = persist.tile([2, 1], f32)
    nc.vector.memset(ones2c[:], 1.0)
    ones32 = persist.tile([32, 1], f32)
    nc.vector.memset(ones32[:], 1.0)

    # ---------------- numerator setup -------
    NQ = T // 16  # 48
    Ywr = persist.tile([128, 8 * NQ], i32)
    iow = persist.tile([128, 8 * NQ], i16)
    nc.gpsimd.iota(iow[:], pattern=[[0, 8], [32, NQ]], base=0, channel_multiplier=0)

    XW = []
    for tau in range(8):
        xw = persist.tile([128, NQ * 32], f32)
        XW.append(xw)

    # flat Y -> pair indices -> DRAM bounce -> wrapped PIDX
    Yi = persist.tile([64, T], i32)
    nc.sync.dma_start(Yi[:], Yd)
    Yf_ = persist.tile([64, T], f32)
    nc.vector.tensor_copy(Yf_[:], Yi[:])
    NP = 800
    pidx = persist.tile([64, NP], f32)
    nc.vector.scalar_tensor_tensor(pidx[:, 0:767], Yf_[:, 0:767], 32.0,
                                   Yf_[:, 1:768], op0=ALU.mult, op1=ALU.add)
    nc.vector.tensor_scalar_add(pidx[:, 767:768], Yf_[:, 0:1], 1024.0)
    nc.vector.tensor_scalar_add(pidx[:, 768:769], Yf_[:, 767:768], 1056.0)
    nc.vector.memset(pidx[:, 769:800], 1088.0)
    pidx16 = persist.tile([64, NP], i16)
    nc.vector.tensor_copy(pidx16[:], pidx[:])
    dpool = es.enter_context(tc.tile_pool(name="dram", bufs=1, space="DRAM"))
    pd = dpool.tile([64, NP], i16)
    pdw = nc.sync.dma_start(pd[:], pidx16[:])
    NPC = NP // 16  # 50
    PIDX = persist.tile([128, 8 * NPC], i16)

    # wrap DMAs (one per batch element) drained a few per scan step so
    # they never crowd out the scan-critical X chunk loads
    wrap_q = []
    for b in range(B):
        g, tau = b % 8, b // 8

        def _ywr(b=b, g=g, tau=tau):
            nc.sync.dma_start(
                Ywr[16 * g:16 * g + 16, NQ * tau:NQ * tau + NQ],
                Yd[b:b + 1, :].rearrange("a (c p) -> a p c", p=16),
            )
        wrap_q.append(_ywr)
    for b in range(B):
        g, tau = b % 8, b // 8

        def _pidxw(b=b, g=g, tau=tau):
            wi = nc.sync.dma_start(
                PIDX[16 * g:16 * g + 16, NPC * tau:NPC * tau + NPC],
                pd[b:b + 1, :].rearrange("a (c p) -> a p c", p=16),
            )
            add_dep_helper(wi.ins, pdw.ins, sync=True,
                           reason="wrap read waits for dram roundtrip write")
        wrap_q.append(_pidxw)

    def build_eidx():
        Ywrf = persist.tile([128, 8 * NQ], f32)
        nc.vector.tensor_copy(Ywrf[:], Ywr[:])
        iowf = persist.tile([128, 8 * NQ], f32)
        nc.vector.tensor_copy(iowf[:], iow[:])
        eidxf = persist.tile([128, 8 * NQ], f32)
        nc.vector.tensor_add(eidxf[:], iowf[:], Ywrf[:])
        EIDX = persist.tile([128, 8 * NQ], i16)
        nc.vector.tensor_copy(EIDX[:], eidxf[:])
        return EIDX

    # table: [T flat 1024 | start 32 | end 32 | zeros 4] on 128 partitions
    TTAB = persist.tile([128, 1092], f32)
    nc.gpsimd.memset(TTAB[0:1, :], 0.0)
    nc.sync.dma_start(TTAB[0:1, 0:1024], Td.rearrange("i j -> (i j)"))
    nc.sync.dma_start(TTAB[0:1, 1024:1056], Sd)
    nc.sync.dma_start(TTAB[0:1, 1056:1088], Ed)
    nc.gpsimd.partition_broadcast(TTAB[:], TTAB[0:1, :])

    # static diag mask for the emission gather: [p, k] = (k%16 == p%16)
    iok = persist.tile([128, T], i16)
    nc.gpsimd.iota(iok[:], pattern=[[0, NQ], [1, 16]], base=0, channel_multiplier=0)
    iokf = persist.tile([128, T], f32)
    nc.vector.tensor_copy(iokf[:], iok[:])
    iop = persist.tile([128, 1], i16)
    nc.gpsimd.iota(iop[:], pattern=[[0, 1]], base=0, channel_multiplier=1)
    pmod = persist.tile([128, 1], i16)
    nc.vector.tensor_scalar(pmod[:], iop[:], 15, None, op0=ALU.bitwise_and)
    pmodf = persist.tile([128, 1], f32)
    nc.vector.tensor_copy(pmodf[:], pmod[:])
    dmask = persist.tile([128, T], f32)
    nc.vector.tensor_scalar(dmask[:], iokf[:], pmodf[:], None, op0=ALU.is_equal)

    # selection matrices for the per-group combine matmuls
    iog = persist.tile([128, 8], i16)
    nc.gpsimd.iota(iog[:], pattern=[[1, 8]], base=0, channel_multiplier=0)
    iogf = persist.tile([128, 8], f32)
    nc.vector.tensor_copy(iogf[:], iog[:])
    pdiv = persist.tile([128, 1], i16)
    nc.vector.tensor_scalar(pdiv[:], iop[:], 4, None, op0=ALU.logical_shift_right)
    pdivf = persist.tile([128, 1], f32)
    nc.vector.tensor_copy(pdivf[:], pdiv[:])
    SELe = persist.tile([128, 8], f32)
    nc.vector.tensor_scalar(SELe[:], iogf[:], pdivf[:], None, op0=ALU.is_equal)
    SELt = persist.tile([128, 8], f32)
    nc.vector.tensor_scalar_mul(SELt[:], SELe[:], 1.0 / 16.0)

    empart = persist.tile([128, 8], f32)
    tpart = persist.tile([128, 8], f32)

    # ---------------- emission pipeline state ----------------
    # paired bf16 X: pair r cols [64r,64r+32) = x_r, [64r+32,64r+64) = x_{767-r}
    XbfP = persist.tile([128, 64 * NPAIR], bf16)
    EX = persist.tile([64, 64 * NPAIR], bf16)
    XbfPv = XbfP[64:128, :].rearrange("p (r h j) -> p r h j", h=2, j=32)

    def chunk_prep_head(i):
        # DMA + exp-cast for chunk i (pairs 32i..32i+31)
        lo = CH * i
        xf = xin.tile([128, CH * 32], f32)
        nc.sync.dma_start(xf[64:128, :], Xf[:, 32 * lo:32 * (lo + CH)])
        nc.scalar.activation(
            XbfPv[:, lo:lo + CH, 0, :],
            xf[64:128, :].rearrange("p (t j) -> p t j", j=32), AF.Exp)
        xb = xin.tile([128, CH * 32], f32)
        blo = T - lo - CH  # covers t' = blo .. blo+CH-1 (= 767-r desc)
        nc.sync.dma_start(xb[64:128, :], Xf[:, 32 * blo:32 * (blo + CH)])
        # reversed read: pair r = 767-t' ascends as t' descends
        nc.scalar.activation(
            XbfPv[:, lo:lo + CH, 1, :],
            xb[64:128, :].rearrange("p (t j) -> p t j", j=32)[:, ::-1, :],
            AF.Exp)

    def emit_pair(r):
        # transpose pair r and exp... (already exp'd) copy PSUM->SBUF
        tp = tp_ps.tile([64, 64], bf16, tag="tp")
        nc.tensor.transpose(tp[:], XbfP[64:128, 64 * r:64 * r + 64],
                            ident[64:128, :])
        nc.scalar.copy(EX[:, 64 * r:64 * r + 64], tp[:])

    # ---------------- numerator gather (spread over chunks) -----------
    def issue_tau(tau):
        for g in range(8):
            b = 8 * tau + g
            nc.sync.dma_start(
                XW[tau][16 * g:16 * g + 16, :],
                Xf[b:b + 1, :].rearrange("a (q p j) -> a p q j", p=16, j=32))

    eidx_box = [None]

    def gather_tau(tau):
        EIDX = eidx_box[0]
        go = gpool.tile([128, T], f32)
        nc.gpsimd.ap_gather(go[:], XW[tau][:], EIDX[:, NQ * tau:NQ * tau + NQ],
                            channels=128, num_elems=NQ * 32, d=1, num_idxs=T)
        junk = gpool.tile([128, T], f32)
        nc.vector.scalar_tensor_tensor(junk[:], go[:], 1.0, dmask[:],
                                       op0=ALU.bypass, op1=ALU.mult,
                                       accum_out=empart[:, tau:tau + 1])
        to = gpool.tile([128, NP], f32)
        nc.gpsimd.ap_gather(to[:], TTAB[:], PIDX[:, NPC * tau:NPC * tau + NPC],
                            channels=128, num_elems=1092, d=1, num_idxs=NP)
        nc.vector.tensor_reduce(tpart[:, tau:tau + 1], to[:], AX.X, ALU.add)

    # ---------------- the scan ----------------
    # +1 slot: a final renorm of the last state keeps the combine's
    # chain-product inside the scalar engine's Ln range
    NREN = len(range(RENORM, NPAIR - REN_LAG, RENORM)) + 1
    rst = persist.tile([2, 64 * NREN], bf16)
    rscr = es.enter_context(tc.tile_pool(name="rscr", bufs=2))
    ren_slot = [0]

    U2 = [persist.tile([64, 64], bf16, name=f"u2_{k}") for k in range(3)]

    def st(r):
        return U2[r % 3]

    def renorm_a(r):
        # measure sums of both chains on state r; scale applied to EX
        # pair r+REN_LAG by renorm_b one step later.  bf16 scales so the
        # applied factor and the Ln-logged factor are bit-identical.
        sp = s_ps.tile([2, 64], f32, tag="sp")
        nc.tensor.matmul(sp[:], ones2[:], st(r)[:], tile_position=(0, 0))
        srec = rscr.tile([2, 64], f32)
        nc.vector.reciprocal_approx_fast(srec[:], sp[:])
        m = ren_slot[0]
        ren_slot[0] += 1
        rsl = rst[:, 64 * m:64 * m + 64]
        nc.vector.tensor_copy(rsl, srec[:])
        return rsl

    def renorm_b(r, rsl):
        rb = rb_ps.tile([64, 64], f32, tag="rb")
        nc.tensor.matmul(rb[:], SEL2T[:], rsl, tile_position=(0, 0))
        ra = r + REN_LAG
        nc.vector.tensor_mul(EX[:, 64 * ra:64 * ra + 64],
                             EX[:, 64 * ra:64 * ra + 64], rb[:])

    # prologue: chunk 0 fully prepped, chunk 1 DMA+cast issued
    chunk_prep_head(0)
    for rr in range(CH):
        emit_pair(rr)
    chunk_prep_head(1)
    emit_pair(CH)  # loop emits r+CH for r>=1, so pair 32 is emitted here

    # u_0 = exp(start) * ex_0 ; y_767 = exp(end) * ex_767
    nc.vector.tensor_scalar_mul(st(0)[:], EX[:, 0:64], expSE[:, 0:1])

    pend = None  # (r, rsl) awaiting renorm_b
    for r in range(1, NPAIR):
        i = r // CH
        if r % CH == 0 and i + 1 < NCHUNK:
            chunk_prep_head(i + 1)
        if r % CH == 1 and 1 <= i <= 8:
            issue_tau(i - 1)
        if r % 8 == 5 and wrap_q:
            for _ in range(8):
                if wrap_q:
                    wrap_q.pop(0)()
        if r == 165:
            eidx_box[0] = build_eidx()
        if r >= 185 and (r - 185) % 16 == 0 and (r - 185) // 16 < 8:
            gather_tau((r - 185) // 16)
        if r + CH < NPAIR:
            emit_pair(r + CH)
        vp = sc_ps.tile([64, 64], f32, tag="sc")
        nc.tensor.matmul(vp[:], W64[:], st(r - 1)[:], tile_position=(0, 0))
        nc.vector.tensor_mul(st(r)[:], vp[:], EX[:, 64 * r:64 * r + 64])
        if pend is not None:
            renorm_b(*pend)
            pend = None
        if r % RENORM == 0 and r + REN_LAG < NPAIR and ren_slot[0] < NREN:
            pend = (r, renorm_a(r))

    # ---------------- combine: Z = u_383^T exp(T) y_384 ----------------
    # final renorm: both chains scaled to unit sum (and logged) so the
    # product stays well inside the Ln table range
    rslF = renorm_a(NPAIR - 1)
    rbF = rb_ps.tile([64, 64], f32, tag="rb")
    nc.tensor.matmul(rbF[:], SEL2T[:], rslF, tile_position=(0, 0))
    last = persist.tile([64, 64], bf16)
    nc.vector.tensor_mul(last[:], st(NPAIR - 1)[:], rbF[:])
    w383 = sc_ps.tile([32, 64], f32, tag="sc")
    nc.tensor.matmul(w383[:], exTT32[32:64, :], last[32:64, :],
                     tile_position=(32, 0))
    q = persist.tile([32, 64], f32)
    nc.vector.tensor_mul(q[:], w383[:], last[0:32, :])
    combo = s_ps.tile([1, 64], f32, tag="sp")
    nc.tensor.matmul(combo[:], ones32[:], q[:], tile_position=(0, 0))

    nump = pp_ps.tile([1, 64], f32, tag="pp")
    for tau in range(8):
        sl = nump[0:1, 8 * tau:8 * tau + 8]
        nc.tensor.matmul(sl, empart[:, tau:tau + 1], SELe[:], start=True,
                         stop=False, tile_position=(0, 0))
        nc.tensor.matmul(sl, tpart[:, tau:tau + 1], SELt[:], start=False,
                         stop=True, tile_position=(0, 0))

    # ---------------- final assembly ----------------
    lncombo = persist.tile([1, 64], f32)
    nc.scalar.activation(lncombo[:], combo[:], AF.Ln)
    lnr = persist.tile([2, 64 * NREN], f32)
    nc.scalar.activation(lnr[:], rst[:], AF.Ln)
    lnrsum = persist.tile([2, 64], f32)
    nc.vector.tensor_reduce(lnrsum[:], lnr[:].rearrange("p (m b) -> p b m", b=64),
                            AX.X, ALU.add)
    lnboth = s_ps.tile([1, 64], f32, tag="sp")
    nc.tensor.matmul(lnboth[:], ones2c[:], lnrsum[:], tile_position=(0, 0))
    f1 = persist.tile([1, 64], f32)
    nc.vector.tensor_sub(f1[:], nump[:], lncombo[:])
    f2 = persist.tile([1, 64], f32)
    nc.vector.tensor_add(f2[:], f1[:], lnboth[:])
    nc.sync.dma_start(Od, f2[:])

    if _DEBUG:
        def dout(name, ap):
            d = nc.dram_tensor(name, list(ap.shape), ap.dtype,
                               kind="ExternalOutput").ap()
            nc.sync.dma_start(d, ap)
        dout("d_ex", EX[:]); dout("d_u2", last[:])
        dout("d_rst", rst[:]); dout("d_q", q[:])
        dout("d_empart", empart[:]); dout("d_tpart", tpart[:])
        dout("d_f1", f1[:]); dout("d_lnrsum", lnrsum[:])

    es.close()


def _build():
    import concourse.tile as tile
    from concourse import bacc, mybir

    f32 = mybir.dt.float32
    i32 = mybir.dt.int32

    nc = bacc.Bacc("TRN2", target_bir_lowering=False, debug=False,
                   enable_asserts=False, num_devices=NCORES)
    Xd = nc.dram_tensor("x", [B, T, NTAG], f32, kind="ExternalInput").ap()
    Yd = nc.dram_tensor("y", [B, T], i32, kind="ExternalInput").ap()
    Td = nc.dram_tensor("t", [NTAG, NTAG], f32, kind="ExternalInput").ap()
    Sd = nc.dram_tensor("s", [NTAG], f32, kind="ExternalInput").ap()
    Ed = nc.dram_tensor("e", [NTAG], f32, kind="ExternalInput").ap()
    Od = nc.dram_tensor("o", [B], f32, kind="ExternalOutput").ap()
    with tile.TileContext(nc) as tc:
        _emit(tc, nc, (Xd, Yd, Td, Sd, Ed, Od))
    nc.compile()
    return nc


def _numpy_fallback(X, Y, mask, transition, start_trans, end_trans):
    X = np.asarray(X, np.float64)
    Y = np.asarray(Y, np.int64)
    m = np.asarray(mask, bool)
    Tm = np.asarray(transition, np.float64)
    st = np.asarray(start_trans, np.float64)
    en = np.asarray(end_trans, np.float64)
    bs, sl, nt = X.shape
    rb = np.arange(bs)
    mf = m.astype(np.float64)
    score = st[Y[:, 0]] + X[rb, 0, Y[:, 0]]
    emit = np.take_along_axis(X[:, 1:], Y[:, 1:, None], axis=2)[..., 0]
    tr = Tm[Y[:, :-1], Y[:, 1:]]
    score = score + np.sum((tr + emit) * mf[:, 1:], axis=1)
    each_len = m.sum(1).astype(np.int64)
    last_tag = Y[rb, each_len - 1]
    score = score + en[last_tag] * mf[rb, each_len - 1]
    alpha = st[None, :] + X[:, 0]
    for t in range(1, sl):
        s = alpha[:, :, None] + Tm[None] + X[:, t][:, None, :]
        mx = s.max(1)
        new = mx + np.log(np.exp(s - mx[:, None, :]).sum(1))
        alpha = np.where(m[:, t][:, None], new, alpha)
    mx = (alpha + en).max(1)
    logZ = mx + np.log(np.exp(alpha + en - mx[:, None]).sum(1))
    return (score - logZ).astype(np.float32)


def kernel(X, Y, mask, transition, start_trans, end_trans):
    X = np.ascontiguousarray(np.asarray(X, dtype=np.float32))
    Yc = np.ascontiguousarray(np.asarray(Y).astype(np.int32))
    Tm = np.ascontiguousarray(np.asarray(transition, dtype=np.float32))
    st = np.ascontiguousarray(np.asarray(start_trans, dtype=np.float32))
    en = np.ascontiguousarray(np.asarray(end_trans, dtype=np.float32))
    mk = np.asarray(mask)

    if X.shape != (BS, T, NTAG) or not bool(mk.all()):
        return _numpy_fallback(X, Y, mask, transition, start_trans, end_trans)

    from concourse import bass_utils

    if "nc" not in _state:
        _state["nc"] = _build()
    nc = _state["nc"]

    in_maps = []
    for c in range(NCORES):
        sl = slice(B * c, B * (c + 1))
        in_maps.append({"x": X[sl], "y": Yc[sl], "t": Tm, "s": st, "e": en})
    res = bass_utils.run_bass_kernel_spmd(nc, in_maps, core_ids=list(range(NCORES)))
    out = np.concatenate([res.results[c]["o"] for c in range(NCORES)])
    return out.astype(np.float32)


if __name__ == "__main__":
    sys.path.insert(0, "/root/problem")
    import reference

    inputs = reference.setup_inputs()
    inputs = {k: np.asarray(v) for k, v in inputs.items()}
    exp = np.asarray(reference.reference(**inputs))
    act = kernel(**inputs)
    err = np.abs(act - exp) / np.maximum(np.abs(exp), 1e-6)
    print("max rel err:", err.max(), "mean:", err.mean())
